# revision 32
# baseline (speedup 1.0000x reference)
"""Trainium2 Bass kernel for nn_CombinedOrthogonalAdapter (MoE-routed LoRA).

Math (per token t): out[t, :] = (x[t, :] @ A_e^T) @ B_e^T,  e = task_indices[t]
with E=8 experts, rank R=64, D=2048, B*S = 16384 tokens, SCALE = 1.0.

Strategy (v4, balanced expert-parallel, host-routed, bf16):
  - Routing is pure data movement, so it happens on host (numpy argsort).
    Core c primarily gets expert c's tokens. Since expert counts exceed
    16384/8 = 2048 on some cores, over-full experts donate their excess
    (<= 128 tokens, one donor per receiver) to under-full cores, so EVERY
    core runs exactly NSLOT = 2048 slots with zero padding. A receiving
    core keeps the foreign tokens in its LAST 128-slot chunk and computes
    them against a second ("foreign") weight set; 0/1 masks shipped from
    the host merge the two rank-64 projections in h-space. If the
    single-donor assignment is infeasible for some input, falls back to
    plain expert-parallel with NSLOT = ceil(max_count/128)*128.
  - Device per core: two dense GEMMs:
      stage A:  H^T[r, s]   = sum_d A[r, d] * xgT[d, s]    (PSUM acc over d)
                (own weights for all slots + foreign weights for the last
                 128 slots; both masked and merged in h-space)
      stage B:  y[s, dout]  = sum_r H^T[r, s] * B[dout, r]
                (last chunk accumulates own + foreign parts in PSUM)
    All matmul inputs bf16 (1 cycle/row on PE), PSUM fp32, evictions cast
    back to bf16. Output y [NSLOT, D] bf16 (one DMA per 128-token slot
    chunk); host scatters tokens back and casts fp32.
  - DMA is the bottleneck: all transfers serialize at ~360 GB/s/core, so
    bf16 I/O and zero slot padding minimize bytes (~17.9 MB/core), and the
    kernel keeps the DMA queue saturated end-to-end: tokens split in two
    column groups so group-0 output DMAs overlap group-1 input DMAs;
    per-chunk evictions alternate DVE/ACT; y_sb is buffered 5 deep so
    buffer-reuse sems never stall the eviction pipeline.
"""

import os

import numpy as np
from ml_dtypes import bfloat16

import concourse.bacc as bacc
import concourse.mybir as mybir
import concourse.tile as tile
from concourse.bass_utils import run_bass_kernel_spmd

# Problem shapes (hardcoded per contest rules).
B, S, D, E, R = 4, 4096, 2048, 8, 64
N_TOK = B * S                     # 16384
N_CORES = 8
CAP = N_TOK // N_CORES            # 2048 slots per core when balanced
DCH = D // 128                    # 16 d chunks

F32 = mybir.dt.float32
BF16 = mybir.dt.bfloat16
MULT = mybir.AluOpType.mult

LAST_RESULTS = None               # test.py introspection hook
_BUILD_CACHE = {}


def _col_tiles(nslot):
    """[(col0, width)] with width <= 512 (one PSUM bank of fp32)."""
    out = []
    c = 0
    while c < nslot:
        w = min(512, nslot - c)
        out.append((c, w))
        c += w
    return out


def _groups(colt):
    """Split col tiles into two pipeline groups of roughly equal width."""
    cut = int(sum(w for _, w in colt) * 0.5)
    acc, g0 = 0, []
    for i, (_, w) in enumerate(colt):
        if g0 and acc + w > cut:
            break
        g0.append(i)
        acc += w
    g1 = [i for i in range(len(colt)) if i not in g0]
    return [g0, g1] if g1 else [g0]


def _build(nslot):
    nc = bacc.Bacc(
        "TRN2",
        target_bir_lowering=False,
        debug=False,
        enable_asserts=False,
        num_devices=N_CORES,
    )

    colt = _col_tiles(nslot)
    groups = _groups(colt)
    wlast = colt[-1][1]
    # stage-A psum: tags shared across groups (one per in-group slot) + the
    # foreign tile; stage-B psum takes the remaining banks (8 total)
    psb_bufs = max(2, 8 - (max(len(g) for g in groups) + 1))

    xgt_d = nc.dram_tensor("xgt", [D, nslot], BF16, kind="ExternalInput")
    # aT packed: ap[p, cd*64 + r] = A_e[r, cd*128 + p]
    a_d = nc.dram_tensor("ap", [128, DCH * R], BF16, kind="ExternalInput")
    # bT: bt[r, dout] = B_e[dout, r]
    b_d = nc.dram_tensor("bt", [R, D], BF16, kind="ExternalInput")
    a2_d = nc.dram_tensor("ap2", [128, DCH * R], BF16, kind="ExternalInput")
    b2_d = nc.dram_tensor("bt2", [R, D], BF16, kind="ExternalInput")
    # masks, broadcast to 64 partitions on host:
    #   cols [0, 128)     own mask over the last 128 slots (mixed chunk)
    #   cols [128, 256)   foreign mask over the last 128 slots
    m_d = nc.dram_tensor("msk", [R, 256], BF16, kind="ExternalInput")
    y_d = nc.dram_tensor("yg", [nslot, D], BF16, kind="ExternalOutput")

    with tile.TileContext(nc) as tc:
        with (
            tc.tile_pool(name="wpool", bufs=1) as wpool,
            tc.tile_pool(name="xpool", bufs=1) as xpool,
            tc.tile_pool(name="hpool", bufs=1) as hpool,
            tc.tile_pool(name="ypool", bufs=1) as ypool,
            tc.tile_pool(name="psA", bufs=1, space="PSUM") as psA,
            tc.tile_pool(name="psB", bufs=psb_bufs, space="PSUM") as psB,
        ):
            a_sb = wpool.tile([128, DCH * R], BF16, name="a_sb", tag="a_sb")
            nc.sync.dma_start(a_sb[:], a_d[:, :])
            b_sb = wpool.tile([R, D], BF16, name="b_sb", tag="b_sb")
            nc.sync.dma_start(b_sb[:], b_d[:, :])
            # group geometry
            gcol = []            # (col0, width) per group
            for g in groups:
                c0 = colt[g[0]][0]
                w = sum(colt[j][1] for j in g)
                gcol.append((c0, w))

            # input DMAs for all groups up-front (program order = DMA order)
            xg_sb = {}
            for gi, g in enumerate(groups):
                c0, gw = gcol[gi]
                for cd in range(DCH):
                    xt = xpool.tile([128, gw], BF16, name=f"x_{gi}_{cd}",
                                    tag=f"x_{gi}_{cd}")
                    nc.sync.dma_start(
                        xt[:], xgt_d[cd * 128:(cd + 1) * 128, c0:c0 + gw])
                    xg_sb[(gi, cd)] = xt

            # foreign weights + masks are needed late; issuing them after the
            # x tiles keeps the early DMA queue fed by larger transfers
            a2_sb = wpool.tile([128, DCH * R], BF16, name="a2_sb", tag="a2_sb")
            nc.sync.dma_start(a2_sb[:], a2_d[:, :])
            b2_sb = wpool.tile([R, D], BF16, name="b2_sb", tag="b2_sb")
            nc.sync.dma_start(b2_sb[:], b2_d[:, :])
            m_sb = wpool.tile([R, 256], BF16, name="m_sb", tag="m_sb")
            nc.sync.dma_start(m_sb[:], m_d[:, :])

            glast = len(groups) - 1
            hF_sb = None
            for gi, g in enumerate(groups):
                c0, gw = gcol[gi]
                # ---- stage A: H^T[r, cols] accumulated over d chunks ----
                hps = {}
                for k, j in enumerate(g):
                    jc0, jw = colt[j]
                    hps[j] = psA.tile([R, jw], F32, name=f"hps{j}",
                                      tag=f"hpsA{k}")
                if gi == glast:
                    hpsF = psA.tile([R, 128], F32, name="hpsF", tag="hpsF")
                for cd in range(DCH):
                    xt = xg_sb[(gi, cd)]
                    for j in g:
                        jc0, jw = colt[j]
                        l0 = jc0 - c0
                        nc.tensor.matmul(
                            hps[j][:],
                            lhsT=a_sb[:, cd * R:(cd + 1) * R],
                            rhs=xt[:, l0:l0 + jw],
                            start=(cd == 0),
                            stop=(cd == DCH - 1),
                        )
                    if gi == glast:
                        # foreign tokens live in the last 128 slots
                        nc.tensor.matmul(
                            hpsF[:],
                            lhsT=a2_sb[:, cd * R:(cd + 1) * R],
                            rhs=xt[:, gw - 128:gw],
                            start=(cd == 0),
                            stop=(cd == DCH - 1),
                        )
                h_sb = hpool.tile([R, gw], BF16, name=f"h_sb{gi}",
                                  tag=f"h_sb{gi}")
                for k, j in enumerate(g):
                    jc0, jw = colt[j]
                    l0 = jc0 - c0
                    if gi == glast and j == g[-1]:
                        # plain copy up to the mixed chunk, then mask out
                        # the foreign columns of the own-weight h
                        if jw > 128:
                            nc.scalar.copy(h_sb[:, l0:l0 + jw - 128],
                                           hps[j][:, 0:jw - 128])
                        nc.vector.tensor_tensor(
                            out=h_sb[:, l0 + jw - 128:l0 + jw],
                            in0=hps[j][:, jw - 128:jw],
                            in1=m_sb[:, 0:128], op=MULT)
                    elif k % 2 == 0:
                        nc.vector.tensor_copy(h_sb[:, l0:l0 + jw], hps[j][:])
                    else:
                        nc.scalar.copy(h_sb[:, l0:l0 + jw], hps[j][:])
                if gi == glast:
                    hF_sb = hpool.tile([R, 128], BF16, name="hF_sb",
                                       tag="hF_sb")
                    nc.vector.tensor_tensor(
                        out=hF_sb[:], in0=hpsF[:],
                        in1=m_sb[:, 128:256], op=MULT)

                # ---- stage B: y[slot, dout] per 128-token slot chunk ----
                # the mixed chunk (extra matmul + mask deps) goes first so
                # its longer latency hides behind queued output DMAs
                nchunk = gw // 128
                sc_order = ([nchunk - 1] + list(range(nchunk - 1))
                            if gi == glast else list(range(nchunk)))
                for sc in sc_order:
                    s0 = c0 + sc * 128          # global slot base
                    l0 = sc * 128               # group-local slot base
                    mixed = (gi == glast and sc == nchunk - 1)
                    y_sb = ypool.tile([128, D], BF16, name="y_sb",
                                      tag="y_sb", bufs=5)
                    # evictions of one chunk alternate DVE/ACT so they run
                    # in parallel and keep the out-DMA fed (GPSIMD cannot
                    # read PSUM)
                    for k in range(D // 512):
                        yps = psB.tile([128, 512], F32, name="yps", tag="yps")
                        nc.tensor.matmul(
                            yps[:],
                            lhsT=h_sb[:, l0:l0 + 128],
                            rhs=b_sb[:, k * 512:(k + 1) * 512],
                            start=True,
                            stop=not mixed,
                        )
                        if mixed:
                            nc.tensor.matmul(
                                yps[:],
                                lhsT=hF_sb[:],
                                rhs=b2_sb[:, k * 512:(k + 1) * 512],
                                start=False,
                                stop=True,
                            )
                        if k % 2 == 0:
                            nc.vector.tensor_copy(
                                y_sb[:, k * 512:(k + 1) * 512], yps[:])
                        else:
                            nc.scalar.copy(
                                y_sb[:, k * 512:(k + 1) * 512], yps[:])
                    nc.sync.dma_start(y_d[s0:s0 + 128, :], y_sb[:])
    nc.compile()
    return nc


def _route(task_indices):
    """Host-side routing: per-expert token index lists (stable order)."""
    idx = np.asarray(task_indices).reshape(-1).astype(np.int64)
    order = np.argsort(idx, kind="stable")
    sorted_idx = idx[order]
    starts = np.searchsorted(sorted_idx, np.arange(E + 1))
    return [order[starts[e]:starts[e + 1]] for e in range(E)]


def _balance(perms):
    """Assign each over-CAP expert's excess to under-full cores: at most
    128 foreign tokens per receiver, all from ONE donor. Returns
    (own, foreign, fexp, nslot) or the unbalanced fallback."""
    own = [p[:CAP] for p in perms]
    foreign = [p[:0] for p in perms]
    fexp = list(range(E))
    ok = True
    for donor in range(E):
        toks = perms[donor][CAP:]
        pos = 0
        for r in range(E):
            if pos >= len(toks):
                break
            if r == donor or len(foreign[r]) > 0:
                continue
            if len(own[r]) < CAP - 128:
                # foreign tokens must land inside the last 128-slot chunk
                continue
            room = min(CAP - len(own[r]), 128)
            take = min(room, len(toks) - pos)
            if take <= 0:
                continue
            foreign[r] = toks[pos:pos + take]
            fexp[r] = donor
            pos += take
        if pos < len(toks):
            ok = False
            break
    if ok:
        return own, foreign, fexp, CAP
    # fallback: plain expert-parallel with padding
    max_cnt = max(len(p) for p in perms)
    nslot = max(((max_cnt + 127) // 128) * 128, CAP)
    return list(perms), [p[:0] for p in perms], list(range(E)), nslot


def prepare_in_maps(x, lora_A, lora_B, task_indices):
    xf = np.asarray(x, dtype=np.float32).reshape(N_TOK, D)
    lora_A = np.asarray(lora_A, dtype=np.float32)
    lora_B = np.asarray(lora_B, dtype=np.float32)
    perms = _route(task_indices)
    own, foreign, fexp, nslot = _balance(perms)

    def pack_a(e):
        return np.ascontiguousarray(
            lora_A[e].T.reshape(DCH, 128, R).transpose(1, 0, 2)
            .reshape(128, DCH * R).astype(bfloat16))

    def pack_b(e):
        return np.ascontiguousarray(lora_B[e].T.astype(bfloat16))

    in_maps = []
    for e in range(E):
        po, pf = own[e], foreign[e]
        xg = np.zeros((nslot, D), dtype=bfloat16)
        xg[:len(po)] = xf[po]
        xg[len(po):len(po) + len(pf)] = xf[pf]
        xgt = np.ascontiguousarray(xg.T)                    # [D, nslot]
        # masks: foreign tokens occupy slots [len(po), len(po)+len(pf)),
        # which lie inside the last 128 slots when balanced
        m = np.zeros((1, 256), dtype=np.float32)
        chunk0 = nslot - 128                                # mixed chunk base
        is_f = np.zeros(nslot, dtype=bool)
        is_f[len(po):len(po) + len(pf)] = True
        m[0, 0:128] = ~is_f[chunk0:nslot]
        m[0, 128:256] = is_f[chunk0:nslot]
        msk = np.ascontiguousarray(
            np.broadcast_to(m, (R, 256)).astype(bfloat16))
        in_maps.append({
            "xgt": xgt,
            "ap": pack_a(e), "bt": pack_b(e),
            "ap2": pack_a(fexp[e]), "bt2": pack_b(fexp[e]),
            "msk": msk,
        })
    return in_maps, own, foreign, nslot


_LAST_NSLOT = CAP


def _get_nc(nslot=None):
    if nslot is None:
        nslot = _LAST_NSLOT
    if nslot not in _BUILD_CACHE:
        _BUILD_CACHE[nslot] = _build(nslot)
    return _BUILD_CACHE[nslot]


def kernel(x, lora_A, lora_B, task_indices):
    global LAST_RESULTS, _LAST_NSLOT
    in_maps, own, foreign, nslot = prepare_in_maps(
        x, lora_A, lora_B, task_indices)
    _LAST_NSLOT = nslot
    nc = _get_nc(nslot)
    res = run_bass_kernel_spmd(
        nc, in_maps, core_ids=list(range(N_CORES)),
        trace=bool(int(os.environ.get("KERNEL_TRACE", "0"))),
    )
    LAST_RESULTS = res

    out = np.empty((N_TOK, D), dtype=np.float32)
    for e in range(E):
        po, pf = own[e], foreign[e]
        yg = np.asarray(res.results[e]["yg"])               # [nslot, D] bf16
        out[po] = yg[:len(po)].astype(np.float32)
        if len(pf):
            out[pf] = yg[len(po):len(po) + len(pf)].astype(np.float32)
    return out.reshape(B, S, D)
